# revision 22
# baseline (speedup 1.0000x reference)
"""Trainium2 Bass kernel for nn_Cross_SelfAttention (B=2, C=256, H=W=64, DQ=16).

Sharding: 8 cores = (batch b in {0,1}) x (attn stream s in {0,1}) x
(query half h in {0,1}).  Each core computes, for its (b, s):
    q = Wq @ x_s + bq   (only its query half i)
    k = Wk @ x_s        (bk dropped: constant-in-j terms cancel in softmax)
    S^T[j, i] = k[:, j] . q[:, i]
    E = exp(S^T) (no max subtraction; |S| <= ~15 so fp32/bf16 exp is safe)
    acc[st] = V_st^T-weighted sums of E columns (st = v1/v2 x 2 c-chunks)
    rowsum  = ones-stationary matmul over the same E
    o = (gamma*Wpt) @ ocat + bpt_eff  (bias via k=1 ones-row matmul)
    out = o * recip(rowsum) + x_residual
bv is folded into bpt_eff on the host (normalization makes the missing
V-bias contribution exactly Wpt @ [bv; bv]); gamma is folded into Wpt/bpt.

Each core writes a disjoint [256, 2048] slice of the output; no
collectives needed.
"""

import numpy as np
import ml_dtypes

import concourse.bass as bass
import concourse.bacc as bacc
import concourse.mybir as mybir
from concourse.tile import TileContext
from concourse.bass import ts

BF16 = mybir.dt.bfloat16
F32 = mybir.dt.float32
F32R = mybir.dt.float32r

def _r(ap):
    """View an fp32 AP as float32r for full-rate PE matmuls (N>=256)."""
    return ap.bitcast(F32R)

B, C, HW, DQ = 2, 256, 4096, 16
HALF = HW // 2          # query positions per core
IB = 512                # i-block size (one PSUM bank at fp32)
N_IB = HALF // IB       # 4 i-blocks
N_JC = HW // 128        # 32 j-chunks

_NC_CACHE = None


def build_bass():
    global _NC_CACHE
    if _NC_CACHE is not None:
        return _NC_CACHE

    nc = bacc.Bacc("TRN2", target_bir_lowering=False, debug=False, num_devices=8)

    # Per-core inputs (full K/V range, query-half for q/residual).
    xq32_d = nc.dram_tensor("xq32", [C, HALF], F32, kind="ExternalInput")
    xk_d = nc.dram_tensor("xk32", [C, HW], F32, kind="ExternalInput")
    xv1_d = nc.dram_tensor("xv1", [C, HW], BF16, kind="ExternalInput")
    xv2_d = nc.dram_tensor("xv2", [C, HW], BF16, kind="ExternalInput")
    wq_d = nc.dram_tensor("wqT", [C, DQ], F32, kind="ExternalInput")
    wk_d = nc.dram_tensor("wkT", [C, DQ], F32, kind="ExternalInput")
    wv_d = nc.dram_tensor("wvT", [C, C], BF16, kind="ExternalInput")
    wpt_d = nc.dram_tensor("wptT", [2 * C, C], F32R, kind="ExternalInput")
    bq_d = nc.dram_tensor("bq_row", [1, DQ], F32, kind="ExternalInput")
    bpt_d = nc.dram_tensor("bpt_col", [128, 2], F32, kind="ExternalInput")
    out_d = nc.dram_tensor("out", [C, HALF], F32, kind="ExternalOutput")

    with TileContext(nc) as tc:
        with (
            tc.tile_pool(name="persist", bufs=1) as pp,
            tc.tile_pool(name="work", bufs=1) as wp,
            tc.tile_pool(name="psum", bufs=1, space="PSUM") as psp,
        ):
            # ---- persistent SBUF tensors ----
            xq32 = pp.tile([128, 2, HALF], F32, name="xq32_sb")
            xk = pp.tile([128, 2, HW], F32, name="xk_sb")
            xv = [
                pp.tile([128, 2, HW], BF16, name=f"xv{r}_sb", tag=f"xv{r}")
                for r in range(2)
            ]
            wq = pp.tile([128, 2, DQ], F32, name="wq_sb")
            wk = pp.tile([128, 2, DQ], F32, name="wk_sb")
            wv = pp.tile([128, 2, C], BF16, name="wv_sb")
            wpt = pp.tile([128, 4, C], F32R, name="wpt_sb")
            bq = pp.tile([1, DQ], F32, name="bq_sb")
            bpt = pp.tile([128, 2], F32, name="bpt_sb")
            ones_row = pp.tile([1, IB], F32, name="ones_row")
            ones128 = pp.tile([128, 128], BF16, name="ones128")
            qsb = pp.tile([DQ, HALF], F32R, name="qsb")
            ksb = pp.tile([DQ, HW], F32R, name="ksb")
            vt = [
                pp.tile([128, N_JC, C], BF16, name=f"vt{r}_sb", tag=f"vt{r}")
                for r in range(2)
            ]

            nc.vector.memset(ones_row[:], 1.0)
            nc.vector.memset(ones128[:], 1.0)

            # ---- load weights + x ----
            r128 = lambda ap: ap.rearrange("(o p) f -> p o f", p=128)
            nc.sync.dma_start(wq[:], r128(wq_d))
            nc.sync.dma_start(wk[:], r128(wk_d))
            nc.sync.dma_start(wv[:], r128(wv_d))
            nc.sync.dma_start(wpt[:], r128(wpt_d))
            nc.sync.dma_start(bq[:], bq_d[:])
            nc.sync.dma_start(bpt[:], bpt_d[:])
            nc.sync.dma_start(xq32[:], r128(xq32_d))
            nc.sync.dma_start(xk[:], r128(xk_d))
            nc.sync.dma_start(xv[0][:], r128(xv1_d))
            nc.sync.dma_start(xv[1][:], r128(xv2_d))

            # ---- Q projection (with bias), K projection (no bias) ----
            for p4 in range(N_IB):
                q_ps = psp.tile([128, IB], F32, name="q_ps", tag="s", bufs=2)
                nc.tensor.matmul(
                    q_ps[:DQ], wq[:, 0], xq32[:, 0, ts(p4, IB)],
                    start=True, stop=False,
                )
                nc.tensor.matmul(
                    q_ps[:DQ], wq[:, 1], xq32[:, 1, ts(p4, IB)],
                    start=False, stop=False,
                )
                nc.tensor.matmul(
                    q_ps[:DQ], bq[:], ones_row[:], start=False, stop=True,
                )
                nc.vector.tensor_copy(qsb[:, ts(p4, IB)], q_ps[:DQ])
            for p8 in range(HW // IB):
                k_ps = psp.tile([128, IB], F32, name="k_ps", tag="s", bufs=2)
                nc.tensor.matmul(
                    k_ps[:DQ], wk[:, 0], xk[:, 0, ts(p8, IB)],
                    start=True, stop=False,
                )
                nc.tensor.matmul(
                    k_ps[:DQ], wk[:, 1], xk[:, 1, ts(p8, IB)],
                    start=False, stop=True,
                )
                nc.vector.tensor_copy(ksb[:, ts(p8, IB)], k_ps[:DQ])

            # ---- V^T projection: vt[r][j, c] (j on partitions) ----
            for r in range(2):
                for jc in range(N_JC):
                    v_ps = psp.tile([128, IB], F32, name="v_ps", tag="s", bufs=2)
                    nc.tensor.matmul(
                        v_ps[:, :C], xv[r][:, 0, ts(jc, 128)], wv[:, 0],
                        start=True, stop=False,
                    )
                    nc.tensor.matmul(
                        v_ps[:, :C], xv[r][:, 1, ts(jc, 128)], wv[:, 1],
                        start=False, stop=True,
                    )
                    nc.vector.tensor_copy(vt[r][:, jc], v_ps[:, :C])

            # ---- main attention loop over i-blocks ----
            for ib in range(N_IB):
                accs = [
                    psp.tile([128, IB], F32, name=f"acc{st}", tag="acc", bufs=5)
                    for st in range(4)
                ]
                acc1 = psp.tile([128, IB], F32, name="acc_ones", tag="acc", bufs=5)
                for jc in range(N_JC):
                    s_ps = psp.tile([128, IB], F32, name="s_ps", tag="s", bufs=2)
                    nc.tensor.matmul(
                        s_ps[:], ksb[:, ts(jc, 128)], qsb[:, ts(ib, IB)],
                        start=True, stop=True,
                    )
                    e_t = wp.tile([128, IB], BF16, name="e_t", tag="E", bufs=3)
                    nc.scalar.activation(
                        e_t[:], s_ps[:], mybir.ActivationFunctionType.Exp
                    )
                    for st in range(4):
                        nc.tensor.matmul(
                            accs[st][:],
                            vt[st // 2][:, jc, ts(st % 2, 128)],
                            e_t[:],
                            start=(jc == 0), stop=(jc == N_JC - 1),
                        )
                    nc.tensor.matmul(
                        acc1[:], ones128[:], e_t[:],
                        start=(jc == 0), stop=(jc == N_JC - 1),
                    )

                r_t = wp.tile([128, IB], F32, name="r_t", tag="R", bufs=2)
                nc.vector.reciprocal(r_t[:], acc1[:])
                ocat = wp.tile([128, 4, IB], F32R, name="ocat", tag="ocat", bufs=2)
                for st in range(4):
                    nc.vector.tensor_copy(ocat[:, st], accs[st][:])

                for cc in range(2):
                    p_ps = psp.tile([128, IB], F32, name="p_ps", tag="proj", bufs=1)
                    for cp in range(4):
                        nc.tensor.matmul(
                            p_ps[:], wpt[:, cp, ts(cc, 128)], ocat[:, cp],
                            start=(cp == 0), stop=(cp == 3),
                        )
                    o_t = wp.tile([128, IB], F32, name="o_t", tag="osb", bufs=3)
                    nc.vector.tensor_mul(o_t[:], p_ps[:], r_t[:])
                    # (o + bpt_eff) + x_residual; bpt is a per-partition scalar
                    nc.vector.scalar_tensor_tensor(
                        o_t[:], o_t[:], bpt[:, cc:cc + 1], xq32[:, cc, ts(ib, IB)],
                        op0=mybir.AluOpType.add, op1=mybir.AluOpType.add,
                    )
                    nc.sync.dma_start(
                        out_d.rearrange("(o p) f -> p o f", p=128)[:, cc, ts(ib, IB)],
                        o_t[:],
                    )

    nc.compile()
    _NC_CACHE = nc
    return nc


def _prep_maps(x, Wq, bq, Wk, bk, Wv, bv, Wpt, bpt, gamma):
    bf16 = ml_dtypes.bfloat16
    f32 = np.float32
    g = float(np.asarray(gamma).reshape(-1)[0])
    wqT = np.ascontiguousarray(Wq.T.astype(f32))
    wkT = np.ascontiguousarray(Wk.T.astype(f32))
    wvT = np.ascontiguousarray(Wv.T.astype(bf16))
    wptT = np.ascontiguousarray((g * Wpt).T.astype(f32))
    bpt_eff = (g * (bpt + Wpt @ np.concatenate([bv, bv]))).astype(np.float32)
    bpt_col = np.ascontiguousarray(bpt_eff.reshape(2, 128).T)
    bq_row = bq.astype(f32).reshape(1, DQ)

    xf = np.asarray(x, np.float32).reshape(B, 2, C, HW)
    x16 = xf.astype(bf16)
    in_maps = []
    for core in range(8):
        b, s, h = core >> 2, (core >> 1) & 1, core & 1
        in_maps.append(
            dict(
                xq32=np.ascontiguousarray(xf[b, s, :, h * HALF:(h + 1) * HALF]),
                xk32=np.ascontiguousarray(xf[b, s]),
                xv1=np.ascontiguousarray(x16[b, 0]),
                xv2=np.ascontiguousarray(x16[b, 1]),
                wqT=wqT, wkT=wkT, wvT=wvT, wptT=wptT,
                bq_row=bq_row, bpt_col=bpt_col,
            )
        )
    return in_maps


def kernel(x, Wq, bq, Wk, bk, Wv, bv, Wpt, bpt, gamma, _trace=False):
    from concourse.bass_utils import run_bass_kernel_spmd

    nc = build_bass()
    in_maps = _prep_maps(x, Wq, bq, Wk, bk, Wv, bv, Wpt, bpt, gamma)
    res = run_bass_kernel_spmd(nc, in_maps, list(range(8)), trace=_trace)

    out = np.empty((B, 2, C, HW), np.float32)
    for core in range(8):
        b, s, h = core >> 2, (core >> 1) & 1, core & 1
        out[b, s, :, h * HALF:(h + 1) * HALF] = res.results[core]["out"]
    full = out.reshape(B, 2 * C, 64, 64)
    if _trace:
        return full, res
    return full


# revision 26
# speedup vs baseline: 3.8501x; 3.8501x over previous
"""Trainium2 Bass kernel for nn_Cross_SelfAttention (B=2, C=256, H=W=64, DQ=16).

Sharding: 8 cores = (batch b in {0,1}) x (attn stream s in {0,1}) x
(query half h in {0,1}).  Each core computes, for its (b, s):
    q = Wq @ x_s + bq   (only its query half i)
    k = Wk @ x_s        (bk dropped: constant-in-j terms cancel in softmax)
    S^T[j, i] = k[:, j] . q[:, i]
    E = exp(S^T) (no max subtraction; |S| <= ~15 so fp32/bf16 exp is safe)
    acc[st] = V_st^T-weighted sums of E columns (st = v1/v2 x 2 c-chunks)
    rowsum  = ones-stationary matmul over the same E
    o = (gamma*Wpt) @ ocat + bpt_eff  (bias via k=1 ones-row matmul)
    out = o * recip(rowsum) + x_residual
bv is folded into bpt_eff on the host (normalization makes the missing
V-bias contribution exactly Wpt @ [bv; bv]); gamma is folded into Wpt/bpt.

Each core writes a disjoint [256, 2048] slice of the output; no
collectives needed.
"""

import os

import numpy as np
import ml_dtypes

import concourse.bass as bass
import concourse.bacc as bacc
import concourse.mybir as mybir
from concourse.tile import TileContext
from concourse.bass import ts

BF16 = mybir.dt.bfloat16
F32 = mybir.dt.float32
F32R = mybir.dt.float32r

def _r(ap):
    """View an fp32 AP as float32r for full-rate PE matmuls (N>=256)."""
    return ap.bitcast(F32R)

B, C, HW, DQ = 2, 256, 4096, 16
HALF = HW // 2          # query positions per core
IB = 512                # i-block size (one PSUM bank at fp32)
N_IB = HALF // IB       # 4 i-blocks
N_JC = HW // 128        # 32 j-chunks

_NC_CACHE = {}

# Debug knob: repeat the main attention loop KREP times inside the program
# (device-time slope measurement through constant dispatch overhead).
KREP = int(os.environ.get("KREP", "1"))


def build_bass():
    if KREP in _NC_CACHE:
        return _NC_CACHE[KREP]

    nc = bacc.Bacc("TRN2", target_bir_lowering=False, debug=False, num_devices=8)

    # Per-core inputs (full K/V range, query-half for q/residual).
    xq32_d = nc.dram_tensor("xq32", [C, HALF], F32, kind="ExternalInput")
    xk_d = nc.dram_tensor("xk32", [C, HW], F32, kind="ExternalInput")
    xv1_d = nc.dram_tensor("xv1", [C, HW], BF16, kind="ExternalInput")
    xv2_d = nc.dram_tensor("xv2", [C, HW], BF16, kind="ExternalInput")
    wq_d = nc.dram_tensor("wqT", [C, DQ], F32, kind="ExternalInput")
    wk_d = nc.dram_tensor("wkT", [C, DQ], F32, kind="ExternalInput")
    wv_d = nc.dram_tensor("wvT", [C, C], BF16, kind="ExternalInput")
    wpt_d = nc.dram_tensor("wptT", [2 * C, C], F32R, kind="ExternalInput")
    bq_d = nc.dram_tensor("bq_row", [1, DQ], F32, kind="ExternalInput")
    bpt_d = nc.dram_tensor("bpt_col", [128, 2], F32, kind="ExternalInput")
    out_d = nc.dram_tensor("out", [C, HALF], F32, kind="ExternalOutput")

    with TileContext(nc) as tc:
        with (
            tc.tile_pool(name="persist", bufs=1) as pp,
            tc.tile_pool(name="work", bufs=1) as wp,
            tc.tile_pool(name="psum", bufs=1, space="PSUM") as psp,
        ):
            # ---- persistent SBUF tensors ----
            xq32 = pp.tile([128, 2, HALF], F32, name="xq32_sb")
            xk = pp.tile([128, 2, HW], F32, name="xk_sb")
            xv = [
                pp.tile([128, 2, HW], BF16, name=f"xv{r}_sb", tag=f"xv{r}")
                for r in range(2)
            ]
            wq = pp.tile([128, 2, DQ], F32, name="wq_sb")
            wk = pp.tile([128, 2, DQ], F32, name="wk_sb")
            wv = pp.tile([128, 2, C], BF16, name="wv_sb")
            wpt = pp.tile([128, 4, C], F32R, name="wpt_sb")
            bq = pp.tile([1, DQ], F32, name="bq_sb")
            bpt = pp.tile([128, 2], F32, name="bpt_sb")
            ones_row = pp.tile([1, IB], F32, name="ones_row")
            ones128 = pp.tile([128, 128], BF16, name="ones128")
            qsb = pp.tile([DQ, HALF], F32R, name="qsb")
            ksb = pp.tile([DQ, HW], F32R, name="ksb")
            vt = [
                pp.tile([128, N_JC, C], BF16, name=f"vt{r}_sb", tag=f"vt{r}")
                for r in range(2)
            ]

            nc.vector.memset(ones_row[:], 1.0)
            nc.vector.memset(ones128[:], 1.0)

            # ---- load weights + x ----
            r128 = lambda ap: ap.rearrange("(o p) f -> p o f", p=128)
            nc.sync.dma_start(wq[:], r128(wq_d))
            nc.sync.dma_start(wk[:], r128(wk_d))
            nc.sync.dma_start(wv[:], r128(wv_d))
            nc.sync.dma_start(wpt[:], r128(wpt_d))
            nc.sync.dma_start(bq[:], bq_d[:])
            nc.sync.dma_start(bpt[:], bpt_d[:])
            nc.sync.dma_start(xq32[:], r128(xq32_d))
            nc.sync.dma_start(xk[:], r128(xk_d))
            nc.sync.dma_start(xv[0][:], r128(xv1_d))
            nc.sync.dma_start(xv[1][:], r128(xv2_d))

            # ---- Q projection (with bias), K projection (no bias) ----
            for p4 in range(N_IB):
                q_ps = psp.tile([128, IB], F32, name="q_ps", tag="s", bufs=2)
                nc.tensor.matmul(
                    q_ps[:DQ], wq[:, 0], xq32[:, 0, ts(p4, IB)],
                    start=True, stop=False,
                )
                nc.tensor.matmul(
                    q_ps[:DQ], wq[:, 1], xq32[:, 1, ts(p4, IB)],
                    start=False, stop=False,
                )
                nc.tensor.matmul(
                    q_ps[:DQ], bq[:], ones_row[:], start=False, stop=True,
                )
                nc.vector.tensor_copy(qsb[:, ts(p4, IB)], q_ps[:DQ])
            for p8 in range(HW // IB):
                k_ps = psp.tile([128, IB], F32, name="k_ps", tag="s", bufs=2)
                nc.tensor.matmul(
                    k_ps[:DQ], wk[:, 0], xk[:, 0, ts(p8, IB)],
                    start=True, stop=False,
                )
                nc.tensor.matmul(
                    k_ps[:DQ], wk[:, 1], xk[:, 1, ts(p8, IB)],
                    start=False, stop=True,
                )
                nc.vector.tensor_copy(ksb[:, ts(p8, IB)], k_ps[:DQ])

            # ---- V^T projection: vt[r][j, c] (j on partitions) ----
            for r in range(2):
                for jc in range(N_JC):
                    v_ps = psp.tile([128, IB], F32, name="v_ps", tag="s", bufs=2)
                    nc.tensor.matmul(
                        v_ps[:, :C], xv[r][:, 0, ts(jc, 128)], wv[:, 0],
                        start=True, stop=False,
                    )
                    nc.tensor.matmul(
                        v_ps[:, :C], xv[r][:, 1, ts(jc, 128)], wv[:, 1],
                        start=False, stop=True,
                    )
                    nc.vector.tensor_copy(vt[r][:, jc], v_ps[:, :C])

            # ---- main attention loop over i-blocks ----
            for ib in [i for _ in range(KREP) for i in range(N_IB)]:
                accs = [
                    psp.tile([128, IB], F32, name=f"acc{st}", tag="acc", bufs=5)
                    for st in range(4)
                ]
                acc1 = psp.tile([128, IB], F32, name="acc_ones", tag="acc", bufs=5)
                for jc in range(N_JC):
                    s_ps = psp.tile([128, IB], F32, name="s_ps", tag="s", bufs=2)
                    nc.tensor.matmul(
                        s_ps[:], ksb[:, ts(jc, 128)], qsb[:, ts(ib, IB)],
                        start=True, stop=True,
                    )
                    e_t = wp.tile([128, IB], BF16, name="e_t", tag="E", bufs=3)
                    nc.scalar.activation(
                        e_t[:], s_ps[:], mybir.ActivationFunctionType.Exp
                    )
                    for st in range(4):
                        nc.tensor.matmul(
                            accs[st][:],
                            vt[st // 2][:, jc, ts(st % 2, 128)],
                            e_t[:],
                            start=(jc == 0), stop=(jc == N_JC - 1),
                        )
                    nc.tensor.matmul(
                        acc1[:], ones128[:], e_t[:],
                        start=(jc == 0), stop=(jc == N_JC - 1),
                    )

                r_t = wp.tile([128, IB], F32, name="r_t", tag="R", bufs=2)
                nc.vector.reciprocal(r_t[:], acc1[:])
                ocat = wp.tile([128, 4, IB], F32R, name="ocat", tag="ocat", bufs=2)
                for st in range(4):
                    nc.vector.tensor_copy(ocat[:, st], accs[st][:])

                for cc in range(2):
                    p_ps = psp.tile([128, IB], F32, name="p_ps", tag="proj", bufs=1)
                    for cp in range(4):
                        nc.tensor.matmul(
                            p_ps[:], wpt[:, cp, ts(cc, 128)], ocat[:, cp],
                            start=(cp == 0), stop=(cp == 3),
                        )
                    o_t = wp.tile([128, IB], F32, name="o_t", tag="osb", bufs=3)
                    nc.vector.tensor_mul(o_t[:], p_ps[:], r_t[:])
                    # (o + bpt_eff) + x_residual; bpt is a per-partition scalar
                    nc.vector.scalar_tensor_tensor(
                        o_t[:], o_t[:], bpt[:, cc:cc + 1], xq32[:, cc, ts(ib, IB)],
                        op0=mybir.AluOpType.add, op1=mybir.AluOpType.add,
                    )
                    nc.sync.dma_start(
                        out_d.rearrange("(o p) f -> p o f", p=128)[:, cc, ts(ib, IB)],
                        o_t[:],
                    )

    nc.compile()
    _NC_CACHE[KREP] = nc
    return nc


def _prep_maps(x, Wq, bq, Wk, bk, Wv, bv, Wpt, bpt, gamma):
    bf16 = ml_dtypes.bfloat16
    f32 = np.float32
    g = float(np.asarray(gamma).reshape(-1)[0])
    wqT = np.ascontiguousarray(Wq.T.astype(f32))
    wkT = np.ascontiguousarray(Wk.T.astype(f32))
    wvT = np.ascontiguousarray(Wv.T.astype(bf16))
    wptT = np.ascontiguousarray((g * Wpt).T.astype(f32))
    bpt_eff = (g * (bpt + Wpt @ np.concatenate([bv, bv]))).astype(np.float32)
    bpt_col = np.ascontiguousarray(bpt_eff.reshape(2, 128).T)
    bq_row = bq.astype(f32).reshape(1, DQ)

    xf = np.asarray(x, np.float32).reshape(B, 2, C, HW)
    x16 = xf.astype(bf16)
    in_maps = []
    for core in range(8):
        b, s, h = core >> 2, (core >> 1) & 1, core & 1
        in_maps.append(
            dict(
                xq32=np.ascontiguousarray(xf[b, s, :, h * HALF:(h + 1) * HALF]),
                xk32=np.ascontiguousarray(xf[b, s]),
                xv1=np.ascontiguousarray(x16[b, 0]),
                xv2=np.ascontiguousarray(x16[b, 1]),
                wqT=wqT, wkT=wkT, wvT=wvT, wptT=wptT,
                bq_row=bq_row, bpt_col=bpt_col,
            )
        )
    return in_maps


def kernel(x, Wq, bq, Wk, bk, Wv, bv, Wpt, bpt, gamma, _trace=False):
    from concourse.bass_utils import run_bass_kernel_spmd

    nc = build_bass()
    in_maps = _prep_maps(x, Wq, bq, Wk, bk, Wv, bv, Wpt, bpt, gamma)
    res = run_bass_kernel_spmd(nc, in_maps, list(range(8)), trace=_trace)

    out = np.empty((B, 2, C, HW), np.float32)
    for core in range(8):
        b, s, h = core >> 2, (core >> 1) & 1, core & 1
        out[b, s, :, h * HALF:(h + 1) * HALF] = res.results[core]["out"]
    full = out.reshape(B, 2 * C, 64, 64)
    if _trace:
        return full, res
    return full
